# revision 38
# baseline (speedup 1.0000x reference)
"""Trainium2 Bass kernel for a (quirky) transformer decoder layer.

Problem shapes: B=2, S=2048, D=128, H=8 heads, head_dim=16.
  sa  = attn(q=x_tgt, kv=x_tgt);  r1 = sa @ w1 + b1 + x_tgt
  ca  = attn(q=enc_out, kv=x_tgt); r2 = ca @ w2 + b2 + r1
  ln  = (r2 - mean) / var   (var unbiased, divide by var not std)
  out = relu(ln @ w3 + b3) @ w4 + b4 + r2
(mask_src / mask_tgt are unused by the reference.)

Sharding: 8 cores, query-row sharding (zero communication). Core c handles
batch c//4, query rows [(c%4)*512 : (c%4+1)*512].

Measured HW facts this kernel is shaped around (micro-benched):
 - A [K=128, M=128, N=512] matmul sustains ~320ns regardless of dtype
   (fp8/bf16/DoubleRow run the same columns no faster: the PE is
   power/issue limited). bf16 operands still help: LDWEIGHTS is ~97ns vs
   ~280ns for fp32r, and MAC power drops, easing the power throttle.
 - Small-K matmuls are SLOWER per column (K=16 -> ~558ns), so scores keep
   the full-128-channel contraction via the host-folded A_h = wk_h@wq_h^T
   (scores^T = x_kv @ (A_h @ x_q^T)), which also removes all Q/K
   projections and their PSUM->SBUF copies.
 - exp on ScalarE costs ~1.11us per [128, 1024] tile regardless of dtype;
   Scalar runs ONLY exp (128 activations), everything else lives on DVE.
 - One dma_start costs ~870ns of Sync-engine time and lands on a single
   ~22GB/s queue, so big inputs are chunked and round-robined between the
   Sync and GpSimd queues, critical chunks first.
All activations stay transposed [d, token] on device; the host uploads
x^T directly and un-transposes the output, so the kernel has zero PE
transposes. Softmax skips max-subtraction (exp in fp32: scores reach ~33,
e^33 ~ 2e14 is finite and the denominator ratio is exact); the denominator
rides the packed V as a ones column (col 16 of each 32-col head group,
memset after the V projection) and is broadcast via a selector matmul +
DVE reciprocal.

Schedule: one flat stream of 128 (attn, head, key-tile-pair) steps; each
step is [2 score matmuls -> exp([128,1024] -> bf16) -> 2 PV matmuls], with
PV lagging the exp by LAG=4 steps (eb pool bufs=6) so transient PE work
(G/V setup, normalization) never starves the Scalar. G/V projections and
both attentions' normalization are interleaved into the stream; the
serial tail (last norm + layernorm + FFN) runs in column halves so PE and
DVE pipeline through it.
"""

import numpy as np
import ml_dtypes

import concourse.bass as bass
import concourse.tile as tile
from concourse import mybir
from concourse.bass_utils import run_bass_kernel_spmd

B, S, D, H, HD = 2, 2048, 128, 8, 16
QC = 512  # query rows per core
NCORES = 8
KT = 16  # number of 128-row key tiles
F32 = mybir.dt.float32
F32R = mybir.dt.float32r
BF16 = mybir.dt.bfloat16
AF = mybir.ActivationFunctionType
OP = mybir.AluOpType
BF16NP = ml_dtypes.bfloat16


# ---------------------------------------------------------------- host packing
def _pack32_cols(w, grp):
    """[D, 128]: col 32g+j (j<16) = w[:, j*H + (4*grp+g)], else 0 (col 16 of
    each 32-group is later memset to 1.0 for the softmax denominator)."""
    out = np.zeros((D, 128), np.float32)
    for g in range(4):
        h = 4 * grp + g
        for j in range(HD):
            out[:, 32 * g + j] = w[:, j * H + h]
    return out


def _pack_w12(w, grp):
    """lhsT for the merge projection: row 32c+j = w[j*H + (4*grp+c), :]."""
    out = np.zeros((D, D), np.float32)
    for c in range(4):
        h = 4 * grp + c
        for j in range(HD):
            out[32 * c + j, :] = w[j * H + h, :]
    return out


def _sel_matrix():
    sel = np.zeros((128, 128), np.float32)
    for m in range(128):
        sel[32 * (m // 32) + 16, m] = 1.0
    return sel


def _split_multiwaits(nc):
    """Post-pass for walrus builds that accept only ONE sync-wait per
    instruction: split every instruction carrying N>1 waits into (N-1)
    single-wait NOPs on the same engine placed immediately before it."""
    uid = 0
    for f in nc.m.functions:
        for bb in f.blocks:
            il = bb.instructions
            if not any(
                i.sync_info is not None
                and i.sync_info.on_wait
                and len(i.sync_info.on_wait) > 1
                for i in il
            ):
                continue
            out = []
            for inst in il:
                si = inst.sync_info
                if si is not None and si.on_wait and len(si.on_wait) > 1:
                    waits = list(si.on_wait)
                    for w in waits[:-1]:
                        uid += 1
                        nop = mybir.InstNoOp(
                            name=f"WSPLIT-{uid}",
                            engine=inst.engine,
                            ins=[],
                            outs=[],
                            sync_info=mybir.SyncInfo(on_wait=[w], on_update=[]),
                        )
                        out.append(nop)
                    inst.sync_info = mybir.SyncInfo(
                        on_wait=[waits[-1]], on_update=list(si.on_update)
                    )
                out.append(inst)
            bb.instructions = out
    return nc


# ---------------------------------------------------------------- device build
def build_nc():
    nc = bass.Bass()

    def din(name, shape, dt=F32R):
        return nc.dram_tensor(name, list(shape), dt, kind="ExternalInput")

    xbTb = din("xbTb", (128, 2048), BF16)  # batch x_tgt transposed, bf16
    xqT = din("xqT", (128, 512))  # query slice of x_tgt, transposed (residual)
    xqTb = din("xqTb", (128, 512), BF16)  # same, bf16 (G rhs)
    eoTb = din("eoTb", (128, 512), BF16)  # enc_out query slice, bf16 (G rhs)
    at = din("at", (128, 16 * 128), BF16)  # A_h^T stacked per (attn*8+h)
    wv_st = din("wv_st", (D, 512), BF16)  # [v_selfA|v_selfB|v_crossA|v_crossB]
    w1p = [din(f"w1p{g}", (D, D), BF16) for g in range(2)]
    w2p = [din(f"w2p{g}", (D, D), BF16) for g in range(2)]
    w3 = din("w3", (D, 512), BF16)
    w4r = din("w4r", (128, 512), BF16)  # col block j = w4[128j:128j+128, :]
    selt = din("selt", (128, 128))  # SEL[p, m] = (p == 32*(m//32)+16)
    ones_col = din("ones_col", (128, 1))
    ones_row = din("ones_row", (1, 128))
    b1t = din("b1t", (128, 1), F32)
    b2t = din("b2t", (128, 1), F32)
    b3t = din("b3t", (128, 4), F32)
    b4t = din("b4t", (128, 1), F32)
    y = nc.dram_tensor("y", [128, 512], F32, kind="ExternalOutput")

    with tile.TileContext(nc) as tc:
        with tc.tile_pool(name="persist", bufs=1) as pp:

            def sbuf(name, shape, dt=F32):
                return pp.tile(list(shape), dt, name=name, tag=name)

            _dmaq = [nc.sync, nc.gpsimd]
            _dman = [0]

            def dma(out, in_):
                _dmaq[_dman[0] % 2].dma_start(out=out, in_=in_)
                _dman[0] += 1

            def load(name, dram, shape, dt=F32R):
                t = sbuf(name, shape, dt)
                dma(t[:], dram[:])
                return t

            def load_chunked(name, dram, shape, dt, nchunk):
                t = sbuf(name, shape, dt)
                step = shape[1] // nchunk
                for c in range(nchunk):
                    dma(t[:, c * step : (c + 1) * step],
                        dram[:, c * step : (c + 1) * step])
                return t

            # critical path first: G(0) needs at chunk 0 + xqTb; the first
            # head consumes every V tile, so xbTb + wv come right after
            at_t = sbuf("at", (128, 16 * 128), BF16)
            dma(at_t[:, 0:256], at[:, 0:256])
            xqTb_t = load("xqTb", xqTb, (128, 512), BF16)
            xbTb_t = sbuf("xbTb", (128, 2048), BF16)
            dma(xbTb_t[:, 0:512], xbTb[:, 0:512])
            wv_t = load("wv", wv_st, (D, 512), BF16)
            for c in range(1, 4):
                dma(xbTb_t[:, 512 * c : 512 * (c + 1)],
                    xbTb[:, 512 * c : 512 * (c + 1)])
            for c in range(1, 8):
                dma(at_t[:, 256 * c : 256 * (c + 1)],
                    at[:, 256 * c : 256 * (c + 1)])
            eoTb_t = load("eoTb", eoTb, (128, 512), BF16)
            xqT_t = load_chunked("xqT", xqT, (128, 512), F32R, 2)
            w1p_t = [load(f"w1p{g}", w1p[g], (D, D), BF16) for g in range(2)]
            w2p_t = [load(f"w2p{g}", w2p[g], (D, D), BF16) for g in range(2)]
            w3_t = load("w3", w3, (D, 512), BF16)
            w4_t = load("w4", w4r, (128, 512), BF16)
            sel_t = load("sel", selt, (128, 128))
            onec_t = load("onec", ones_col, (128, 1))
            oner_t = load("oner", ones_row, (1, 128))
            b1_t = load("b1", b1t, (128, 1), F32)
            b2_t = load("b2", b2t, (128, 1), F32)
            b3_t = load("b3", b3t, (128, 4), F32)
            b4_t = load("b4", b4t, (128, 1), F32)

            gs = [sbuf(f"g{ah}", (128, 512), BF16) for ah in range(16)]
            vs = [sbuf(f"v{t}", (128, 512), BF16) for t in range(KT)]

            acc_sb = [[sbuf(f"acc{a}{g}", (128, 512), F32R) for g in range(2)]
                      for a in range(2)]
            r1T = sbuf("r1T", (128, 512))
            r2T = sbuf("r2T", (128, 512), F32R)
            sa_n = {}
            HV = (slice(0, 256), slice(256, 512))

            with tc.tile_pool(name="pattn", bufs=1, space="PSUM") as pa, \
                 tc.tile_pool(name="ebp", bufs=6) as ebp:

                def g_setup(ah):
                    """G_ah = A_ah @ x_q^T -> bf16 SBUF (uses an nm bank,
                    idle until the first normalize)."""
                    xsrc = xqTb_t if ah < 8 else eoTb_t
                    gp = pa.tile([128, 512], F32, name=f"gp{ah}", tag="nm0")
                    nc.tensor.matmul(
                        gp[:],
                        lhsT=at_t[:, 128 * ah : 128 * (ah + 1)],
                        rhs=xsrc[:],
                        start=True,
                        stop=True,
                    )
                    nc.vector.tensor_copy(out=gs[ah][:], in_=gp[:])

                def v_setup(t):
                    vp = pa.tile([128, 512], F32, name=f"vp{t}", tag="nm1")
                    nc.tensor.matmul(
                        vp[:],
                        lhsT=xbTb_t[:, 128 * t : 128 * (t + 1)],
                        rhs=wv_t[:],
                        start=True,
                        stop=True,
                    )
                    nc.vector.tensor_copy(out=vs[t][:], in_=vp[:])
                    # denominator ones columns (after the V copy so they are
                    # not overwritten by the zero-padded projection)
                    nc.vector.memset(
                        vs[t][:].rearrange("p (c x) -> p c x", x=32)[:, :, 16],
                        1.0,
                    )

                def sc_exp(a, h, p):
                    """two score matmuls + one exp for key-tile pair p."""
                    ah = 8 * a + h
                    sc = pa.tile([128, 1024], F32, bufs=2,
                                 name=f"sc{ah}{p}", tag="sc")
                    for i in range(2):
                        nc.tensor.matmul(
                            sc[:, 512 * i : 512 * (i + 1)],
                            lhsT=xbTb_t[
                                :, 128 * (2 * p + i) : 128 * (2 * p + i + 1)
                            ],
                            rhs=gs[ah][:],
                            start=True,
                            stop=True,
                        )
                    eb = ebp.tile([128, 1024], BF16, name="eb", tag="eb")
                    with nc.allow_low_precision(reason="bf16 softmax"):
                        nc.scalar.activation(eb[:], sc[:], AF.Exp, scale=0.25)
                    return eb

                def pv_step(a, h, p, pv, eb):
                    c0 = 256 * a + 128 * (h // 4) + 32 * (h % 4)
                    for i in range(2):
                        nc.tensor.matmul(
                            pv[:],
                            lhsT=vs[2 * p + i][:, c0 : c0 + 32],
                            rhs=eb[:, 512 * i : 512 * (i + 1)],
                            start=(p == 0 and i == 0),
                            stop=(p == 7 and i == 1),
                            skip_group_check=True,
                        )
                    if p == 7:
                        nc.vector.tensor_copy(
                            out=acc_sb[a][h // 4][
                                32 * (h % 4) : 32 * (h % 4) + 32, :
                            ],
                            in_=pv[:],
                        )

                def norm_part1(a, grps=(0, 1)):
                    for g in grps:
                        sbc = pa.tile([128, 512], F32, name=f"sbc{a}{g}",
                                      tag=f"nm{g}")
                        nc.tensor.matmul(sbc[:], lhsT=sel_t[:],
                                         rhs=acc_sb[a][g][:],
                                         start=True, stop=True)
                        rb = pp.tile([128, 512], F32, name=f"rb{a}{g}",
                                     tag=f"rb{g}")
                        nc.vector.reciprocal(out=rb[:], in_=sbc[:])
                        sn = pp.tile([128, 512], BF16, name=f"sn{a}{g}",
                                     tag=f"sn{g}")
                        nc.vector.tensor_mul(sn[:], acc_sb[a][g][:], rb[:])
                        sa_n[(a, g)] = sn

                def norm_part2(a, wp_t):
                    rp = pa.tile([128, 512], F32, name=f"rp{a}", tag="nm0")
                    for g in range(2):
                        nc.tensor.matmul(rp[:], lhsT=wp_t[g][:],
                                         rhs=sa_n[(a, g)][:],
                                         start=(g == 0), stop=(g == 1))
                    nc.vector.scalar_tensor_tensor(
                        out=r1T[:], in0=rp[:], scalar=b1_t[:],
                        in1=xqT_t[:], op0=OP.add, op1=OP.add)

                # flat software-pipelined stream (see module docstring)
                LAG = 4
                g_setup(0)
                v_setup(0)
                v_setup(1)
                steps = [(a, h, p)
                         for a in range(2) for h in range(H) for p in range(8)]
                pvs = {}
                from collections import deque
                pend = deque()
                rp2 = None
                for s, (a, h, p) in enumerate(steps):
                    if p == 0:
                        pvs[(a, h)] = pa.tile([32, 512], F32,
                                              name=f"pv{a}{h}",
                                              tag=f"pv{(8 * a + h) % 2}")
                    pend.append((a, h, p, sc_exp(a, h, p)))
                    if len(pend) > LAG:
                        pa_, ph_, pp_, peb_ = pend.popleft()
                        pv_step(pa_, ph_, pp_, pvs[(pa_, ph_)], peb_)
                    if s < 7:  # V tiles 2..15 during head 0
                        v_setup(2 * s + 2)
                        v_setup(2 * s + 3)
                    elif s < 22:  # remaining G, one per step
                        g_setup(s - 6)
                    if (a, h, p) == (1, 1, 4):
                        norm_part1(0)
                    elif (a, h, p) == (1, 2, 4):
                        norm_part2(0, w1p_t)
                    elif (a, h, p) == (1, 5, 4):
                        norm_part1(1, grps=(0,))
                    elif (a, h, p) == (1, 6, 4):
                        # start the attn-1 merge with group 0 (full width)
                        rp2 = pa.tile([128, 512], F32, name="rp1", tag="nm0")
                        nc.tensor.matmul(rp2[:], lhsT=w2p_t[0][:],
                                         rhs=sa_n[(1, 0)][:],
                                         start=True, stop=False,
                                         skip_group_check=True)
                while pend:
                    pa_, ph_, pp_, peb_ = pend.popleft()
                    pv_step(pa_, ph_, pp_, pvs[(pa_, ph_)], peb_)

                # ------- tail part A (needs the pattn pool): last group's
                # normalize + merge + r2, in column halves so PE/DVE pipeline
                sbc = pa.tile([128, 512], F32, name="sbc11", tag="nm1")
                rb = pp.tile([128, 512], F32, name="rb11", tag="rb1")
                sn = pp.tile([128, 512], BF16, name="sn11", tag="sn1")
                sa_n[(1, 1)] = sn
                for cs in HV:
                    nc.tensor.matmul(sbc[:, cs], lhsT=sel_t[:],
                                     rhs=acc_sb[1][1][:, cs],
                                     start=True, stop=True,
                                     skip_group_check=True)
                    nc.vector.reciprocal(out=rb[:, cs], in_=sbc[:, cs])
                    nc.vector.tensor_mul(sn[:, cs], acc_sb[1][1][:, cs],
                                         rb[:, cs])
                for cs in HV:
                    nc.tensor.matmul(rp2[:, cs], lhsT=w2p_t[1][:],
                                     rhs=sn[:, cs],
                                     start=False, stop=True,
                                     skip_group_check=True)
                    nc.vector.scalar_tensor_tensor(
                        out=r2T[:, cs], in0=rp2[:, cs], scalar=b2_t[:],
                        in1=r1T[:, cs], op0=OP.add, op1=OP.add)

            # ------- tail part B: layernorm (x-m)/var (unbiased) + FFN,
            # staged in column halves
            with tc.tile_pool(name="ptail", bufs=1, space="PSUM") as pt:
                sq = sbuf("sq", (128, 512), F32R)
                mp = pt.tile([1, 512], F32, name="mp", tag="st0")
                sp = pt.tile([1, 512], F32, name="sp", tag="st1")
                msb = sbuf("msb", (1, 512), F32R)
                ssb = sbuf("ssb", (1, 512))
                t0 = sbuf("t0", (1, 512), F32R)
                mbc = pt.tile([128, 512], F32, name="mbc", tag="bc0")
                vbc = pt.tile([128, 512], F32, name="vbc", tag="bc1")
                ivb = sbuf("ivb", (128, 512))
                cT = sbuf("cT", (128, 512))
                lnT = sbuf("lnT", (128, 512), BF16)
                for cs in HV:
                    nc.scalar.square(sq[:, cs], r2T[:, cs])
                    nc.tensor.matmul(mp[:, cs], lhsT=onec_t[:],
                                     rhs=r2T[:, cs], start=True, stop=True,
                                     skip_group_check=True)
                    nc.tensor.matmul(sp[:, cs], lhsT=onec_t[:],
                                     rhs=sq[:, cs], start=True, stop=True,
                                     skip_group_check=True)
                for cs in HV:
                    nc.vector.tensor_scalar_mul(msb[:, cs], mp[:, cs],
                                                1.0 / 128)  # mean
                    nc.vector.tensor_copy(out=ssb[:, cs], in_=sp[:, cs])
                    nc.vector.tensor_mul(t0[:, cs], msb[:, cs], mp[:, cs])
                    nc.vector.tensor_sub(t0[:, cs], ssb[:, cs], t0[:, cs])
                    nc.vector.tensor_scalar_mul(t0[:, cs], t0[:, cs],
                                                1.0 / 127)  # var
                for cs in HV:
                    nc.tensor.matmul(mbc[:, cs], lhsT=oner_t[:],
                                     rhs=msb[:, cs], start=True, stop=True,
                                     skip_group_check=True)
                    nc.tensor.matmul(vbc[:, cs], lhsT=oner_t[:],
                                     rhs=t0[:, cs], start=True, stop=True,
                                     skip_group_check=True)
                for cs in HV:
                    nc.vector.reciprocal(out=ivb[:, cs], in_=vbc[:, cs])
                    nc.vector.tensor_sub(cT[:, cs], r2T[:, cs], mbc[:, cs])
                    nc.vector.tensor_mul(lnT[:, cs], cT[:, cs], ivb[:, cs])

                # ---------------- FFN
                hps = [pt.tile([128, 512], F32, name=f"hp{j}",
                               tag=f"hp{j % 2}") for j in range(4)]
                h_sb = [sbuf(f"hs{j}", (128, 512), BF16) for j in range(4)]
                op_ = pt.tile([128, 512], F32, name="op", tag="bc0")
                oT = sbuf("oT", (128, 512))
                for cs in HV:
                    for j in range(4):
                        nc.tensor.matmul(
                            hps[j][:, cs],
                            lhsT=w3_t[:, 128 * j : 128 * (j + 1)],
                            rhs=lnT[:, cs], start=True, stop=True,
                            skip_group_check=True)
                        nc.scalar.activation(
                            h_sb[j][:, cs], hps[j][:, cs], AF.Relu,
                            bias=b3_t[:, j : j + 1],
                        )
                for cs in HV:
                    for j in range(4):
                        nc.tensor.matmul(
                            op_[:, cs],
                            lhsT=w4_t[:, 128 * j : 128 * (j + 1)],
                            rhs=h_sb[j][:, cs],
                            start=(j == 0), stop=(j == 3),
                            skip_group_check=True)
                    nc.vector.scalar_tensor_tensor(
                        out=oT[:, cs], in0=op_[:, cs], scalar=b4_t[:],
                        in1=r2T[:, cs], op0=OP.add, op1=OP.add)
                    for c in (0, 1):
                        base = cs.start + 128 * c
                        nc.sync.dma_start(out=y[:, base : base + 128],
                                          in_=oT[:, base : base + 128])

    return nc


_CACHED = {}


def _get_nc():
    if "nc" not in _CACHED:
        _CACHED["nc"] = _split_multiwaits(build_nc())
    return _CACHED["nc"]


def _host_inputs(x_tgt, enc_out, self_wq, self_wk, self_wv, cross_wq, cross_wk,
                 cross_wv, w1, b1, w2, b2, w3, b3, w4, b4):
    # folded score matrices A_h = wk_h @ wq_h^T (contract head_dim on host)
    at = np.zeros((2, H, D, D), np.float32)
    for a, (wq, wk) in enumerate(((self_wq, self_wk), (cross_wq, cross_wk))):
        for h in range(H):
            at[a, h] = (wk[:, h::H] @ wq[:, h::H].T).T
    at_flat = np.ascontiguousarray(
        at.reshape(16, D, D).transpose(1, 0, 2).reshape(D, 16 * D)
    )

    shared = {
        "at": at_flat.astype(BF16NP),
        "wv_st": np.concatenate(
            [_pack32_cols(self_wv, 0), _pack32_cols(self_wv, 1),
             _pack32_cols(cross_wv, 0), _pack32_cols(cross_wv, 1)], axis=1
        ).astype(BF16NP),
        "w1p0": _pack_w12(w1, 0).astype(BF16NP),
        "w1p1": _pack_w12(w1, 1).astype(BF16NP),
        "w2p0": _pack_w12(w2, 0).astype(BF16NP),
        "w2p1": _pack_w12(w2, 1).astype(BF16NP),
        "w3": w3.astype(BF16NP),
        "w4r": np.ascontiguousarray(
            w4.reshape(4, 128, 128).transpose(1, 0, 2).reshape(128, 512)
        ).astype(BF16NP),
        "selt": _sel_matrix(),
        "ones_col": np.ones((128, 1), np.float32),
        "ones_row": np.ones((1, 128), np.float32),
        "b1t": b1.reshape(128, 1),
        "b2t": b2.reshape(128, 1),
        "b3t": np.ascontiguousarray(b3.reshape(4, 128).T),
        "b4t": b4.reshape(128, 1),
    }
    shared = {k: (v if v.dtype == BF16NP else
                  np.ascontiguousarray(v, dtype=np.float32))
              for k, v in shared.items()}

    in_maps = []
    for c in range(NCORES):
        b, qb = divmod(c, 4)
        q0 = qb * QC
        im = dict(shared)
        im["xbTb"] = np.ascontiguousarray(x_tgt[b].T).astype(BF16NP)
        im["xqT"] = np.ascontiguousarray(x_tgt[b, q0 : q0 + QC].T)
        im["xqTb"] = im["xqT"].astype(BF16NP)
        im["eoTb"] = np.ascontiguousarray(
            enc_out[b, q0 : q0 + QC].T).astype(BF16NP)
        in_maps.append(im)
    return in_maps


def run_on_device(in_maps, **kw):
    nc = _get_nc()
    return run_bass_kernel_spmd(nc, in_maps, list(range(NCORES)), **kw)


def kernel(x_tgt, enc_out, self_wq, self_wk, self_wv, cross_wq, cross_wk,
           cross_wv, w1, b1, w2, b2, w3, b3, w4, b4, mask_src=None,
           mask_tgt=None, **_unused):
    args = [x_tgt, enc_out, self_wq, self_wk, self_wv, cross_wq, cross_wk,
            cross_wv, w1, b1, w2, b2, w3, b3, w4, b4]
    args = [np.asarray(a, dtype=np.float32) for a in args]
    in_maps = _host_inputs(*args)
    res = run_on_device(in_maps)
    out = np.empty((B, S, D), np.float32)
    for c in range(NCORES):
        b, qb = divmod(c, 4)
        out[b, qb * QC : (qb + 1) * QC] = res.results[c]["y"].T
    return out
